# revision 1
# baseline (speedup 1.0000x reference)
"""BatchGGNNEncoder Trainium2 kernel: 8-core SPMD, dst-sharded message passing.

Full inputs in, full output out. Internally:
  - core c owns nodes [c*4096, (c+1)*4096) = graphs [4c, 4c+4) (data parallel).
  - aggregate-first GGNN layer:
        A_t[v] = sum_{e: dst=v, type=t} h[src_e]         (one-hot matmuls, PSUM)
        m      = sum_t A_t @ Wm[t].T + counts_t * bm[t]  (dense matmuls)
        h      = GRU(m, h)                               (matmuls + DVE/ACT)
  - h table (bf16, node-major) lives in DRAM, AllGathered across cores per layer;
    per-edge h[src] rows fetched with dma_gather.
  - nodes are permuted within each graph to balance (type, 128-dst-window) group
    sizes so the compiled program structure is identical on all 8 cores.
"""
import numpy as np
import ml_dtypes

import concourse.bass as bass
import concourse.bacc as bacc
import concourse.mybir as mybir
import concourse.tile as tile
from concourse.bass_utils import run_bass_kernel_spmd

BF16 = ml_dtypes.bfloat16

# problem constants (hardcoded per harness contract)
MAXN, F, H, T, L = 1024, 215, 256, 8, 3
NCORES = 8
WIN = 128                     # dst window (one-hot free width)
WPG = MAXN // WIN             # 8 windows per graph


def _balance_graph(deg):
    """Assign 1024 nodes (deg: [1024, T] type-degrees) to 8 windows of 128.
    Window WPG-1 takes the heaviest 128 nodes (the graph's excess, ~3 chunks
    per type); the remaining 896 are balanced across windows 0..WPG-2 under a
    hard 256 cap per type (2 chunks), with real slack since the heavy nodes
    are gone. Keeps cross-core max budgets at 2 for most groups."""
    tot = deg.sum(1)
    order = np.argsort(-tot, kind="stable")
    last = WPG - 1
    wsum = np.zeros((WPG, T), np.float64)
    wcnt = np.zeros(WPG, np.int64)
    members = [[] for _ in range(WPG)]
    CAP, CAP7 = 256.0, 381.0
    rest = []
    for nd in order:
        if wcnt[last] < 128 and ((wsum[last] + deg[nd]) <= CAP7).all():
            members[last].append(nd)
            wsum[last] += deg[nd]
            wcnt[last] += 1
        else:
            rest.append(nd)
    for nd in rest:
        d = deg[nd]
        ns = wsum[:last] + d
        feas = (wcnt[:last] < 128) & (ns <= CAP).all(axis=1)
        if feas.any():
            load = np.where(feas, ns.max(axis=1), np.inf)
            best = int(np.argmin(load))
        else:
            nsall = wsum + d
            dcost = (np.ceil(nsall / 128) - np.ceil(wsum / 128)).sum(axis=1)
            dcost[wcnt >= 128] = np.inf
            best = int(np.argmin(dcost))
        members[best].append(nd)
        wsum[best] += d
        wcnt[best] += 1
    return [np.array(m, np.int64) for m in members]


def _prep(node_features, edge_index, edge_type, Wp, bp, Wm, bm, Wih, Whh, bih, bhh):
    """Host-side sharding/packing. Returns (meta, in_maps)."""
    x = np.asarray(node_features, np.float32)
    B = x.shape[0]
    N = B * MAXN
    GPC = B // NCORES             # graphs per core
    NB = GPC * MAXN               # nodes per core
    NWIN = GPC * WPG              # windows per core
    src = np.asarray(edge_index[0]).astype(np.int64)
    dst = np.asarray(edge_index[1]).astype(np.int64)
    et = np.asarray(edge_type).astype(np.int64)

    # per-(node, type) in-degree
    cnt = np.zeros((N, T), np.int64)
    np.add.at(cnt, (dst, et), 1)

    # balance windows within each graph -> node permutation
    old2new = np.empty(N, np.int64)
    for g in range(B):
        mem = _balance_graph(cnt[g * MAXN:(g + 1) * MAXN])
        for w in range(WPG):
            pos = g * MAXN + w * WIN + np.arange(WIN)
            old2new[g * MAXN + mem[w]] = pos
    new2old = np.argsort(old2new)

    src_n = old2new[src]
    dst_n = old2new[dst]

    # group edges per core: key = ((gslot*WPG + w)*T + t)
    core = dst_n // NB
    rel = dst_n % NB
    win_in_core = rel // WIN      # 0..NWIN-1  (gslot*WPG + w)
    col = rel % WIN
    key = win_in_core * T + et
    NGRP = NWIN * T

    gsizes = np.zeros((NCORES, NGRP), np.int64)
    for c in range(NCORES):
        m = core == c
        gsizes[c] = np.bincount(key[m], minlength=NGRP)
    budget = np.ceil(gsizes.max(axis=0) / 128).astype(np.int64)  # chunks per group
    budget = np.maximum(budget, 1)
    ctot = int(budget.sum())
    ngg = (ctot + 7) // 8          # gather groups of 8 chunks
    ctot8 = ngg * 8
    nslots = ctot8 * 128
    gbase = np.concatenate([[0], np.cumsum(budget)])[:-1] * 128  # slot base per group

    # per-core slot arrays
    idx_maps, smat_maps = [], []
    counts_maps, xT_maps = [], []
    for c in range(NCORES):
        m = core == c
        kc, cc, sc = key[m], col[m], src_n[m]
        order = np.argsort(kc, kind="stable")
        kc, cc, sc = kc[order], cc[order], sc[order]
        # rank within group
        grp_start = np.searchsorted(kc, np.arange(NGRP), side="left")
        rank = np.arange(kc.size) - grp_start[kc]
        slot = gbase[kc] + rank
        src16 = np.zeros(nslots, np.int16)
        scol = np.full(nslots, -1, np.int64)
        src16[slot] = sc.astype(np.int16)
        scol[slot] = cc
        # idx: wrapped [16, nslots/16] replicated to 128 partitions
        idx = np.tile(src16.reshape(nslots // 16, 16).T, (8, 1)).copy()
        idx_maps.append(idx)
        # one-hot S: [ngg, 128, 8, 128] bf16
        smat = np.zeros((ctot8 * 128, WIN), BF16)
        valid = scol >= 0
        smat[np.nonzero(valid)[0], scol[valid]] = 1
        smat = smat.reshape(ctot8, 128, WIN).reshape(ngg, 8, 128, WIN)
        smat = np.ascontiguousarray(smat.transpose(0, 2, 1, 3))  # [ngg,128,8,128]
        smat_maps.append(smat)
        # counts (new order), [T, NB] bf16
        cslice = cnt[new2old[c * NB:(c + 1) * NB]]
        counts_maps.append(np.ascontiguousarray(cslice.T).astype(BF16))
        # xT [128, 2, NB] bf16: [p, k, node] = x[node, k*128+p]
        xs = x.reshape(N, F)[new2old[c * NB:(c + 1) * NB]]
        xp = np.zeros((NB, 2 * 128), np.float32)
        xp[:, :F] = xs
        xT = np.ascontiguousarray(xp.reshape(NB, 2, 128).transpose(2, 1, 0))
        xT_maps.append(xT.astype(BF16))

    # weights (shared across cores)
    Wp = np.asarray(Wp, np.float32); bp_ = np.asarray(bp, np.float32)
    Wm_ = np.asarray(Wm, np.float32); bm_ = np.asarray(bm, np.float32)
    Wih_ = np.asarray(Wih, np.float32); Whh_ = np.asarray(Whh, np.float32)
    bih_ = np.asarray(bih, np.float32); bhh_ = np.asarray(bhh, np.float32)

    wpT = np.zeros((128, 2, H), np.float32)          # [p, fk, h']
    wpt = Wp.T                                       # [F, H]
    wpT[:, 0, :] = wpt[0:128]
    wpT[:F - 128, 1, :] = wpt[128:F]
    wp_in = wpT.astype(BF16)
    bp_in = np.ascontiguousarray(bp_.reshape(2, 128).T)          # [128, 2]

    wm_in = np.ascontiguousarray(                     # [L, 128, 2, T, H]
        Wm_.transpose(0, 1, 3, 2)                     # [L,T,h,h']
        .reshape(L, T, 2, 128, H).transpose(0, 3, 2, 1, 4)).astype(BF16)
    bm_in = bm_.astype(BF16)                          # [L, T, H]
    wih_in = np.ascontiguousarray(                    # [L, 128, 2, 3H]
        Wih_.transpose(0, 2, 1).reshape(L, 2, 128, 3 * H).transpose(0, 2, 1, 3)
    ).astype(BF16)
    whh_in = np.ascontiguousarray(
        Whh_.transpose(0, 2, 1).reshape(L, 2, 128, 3 * H).transpose(0, 2, 1, 3)
    ).astype(BF16)
    brz = bih_[:, :2 * H] + bhh_[:, :2 * H]
    brz_in = np.ascontiguousarray(brz.reshape(L, 4, 128).transpose(0, 2, 1))  # [L,128,4]
    bin_in = np.ascontiguousarray(bih_[:, 2 * H:].reshape(L, 2, 128).transpose(0, 2, 1))
    bhn_in = np.ascontiguousarray(bhh_[:, 2 * H:].reshape(L, 2, 128).transpose(0, 2, 1))
    id128 = np.eye(128, dtype=BF16)

    in_maps = []
    for c in range(NCORES):
        in_maps.append({
            "xT": xT_maps[c], "idx": idx_maps[c], "smat": smat_maps[c],
            "countsT": counts_maps[c],
            "wpT": wp_in, "bp": bp_in, "wmT": wm_in, "bmT": bm_in,
            "wihT": wih_in, "whhT": whh_in,
            "brz": brz_in, "bin_": bin_in, "bhn": bhn_in, "id128": id128,
        })
    meta = dict(B=B, N=N, GPC=GPC, NB=NB, NWIN=NWIN,
                budget=budget.reshape(NWIN, T), ctot=ctot, ngg=ngg,
                new2old=new2old)
    return meta, in_maps


def _build(meta, debug=False, skip=()):
    """Build the SPMD Bass program (identical across cores).
    skip: iterable of component names to omit (for timing ablations):
    gather, sload, aggmm, aggcp, wt, gru, gruelt, ag, tr."""
    skip = frozenset(skip)
    dt = mybir.dt
    N, NB, GPC, NWIN = meta["N"], meta["NB"], meta["GPC"], meta["NWIN"]
    budget, ngg = meta["budget"], meta["ngg"]
    ctot8 = ngg * 8
    SLOT16 = ctot8 * 128 // 16

    nc = bacc.Bacc("TRN2", target_bir_lowering=False, debug=False,
                   enable_asserts=False, num_devices=NCORES)

    # ---- I/O
    xT_in = nc.dram_tensor("xT", [128, 2, NB], dt.bfloat16, kind="ExternalInput").ap()
    idx_in = nc.dram_tensor("idx", [128, SLOT16], dt.int16, kind="ExternalInput").ap()
    smat_in = nc.dram_tensor("smat", [ngg, 128, 8, WIN], dt.bfloat16, kind="ExternalInput").ap()
    counts_in = nc.dram_tensor("countsT", [T, NB], dt.bfloat16, kind="ExternalInput").ap()
    wp_in = nc.dram_tensor("wpT", [128, 2, H], dt.bfloat16, kind="ExternalInput").ap()
    bp_in = nc.dram_tensor("bp", [128, 2], dt.float32, kind="ExternalInput").ap()
    wm_in = nc.dram_tensor("wmT", [L, 128, 2, T, H], dt.bfloat16, kind="ExternalInput").ap()
    bm_in = nc.dram_tensor("bmT", [L, T, H], dt.bfloat16, kind="ExternalInput").ap()
    wih_in = nc.dram_tensor("wihT", [L, 128, 2, 3 * H], dt.bfloat16, kind="ExternalInput").ap()
    whh_in = nc.dram_tensor("whhT", [L, 128, 2, 3 * H], dt.bfloat16, kind="ExternalInput").ap()
    brz_in = nc.dram_tensor("brz", [L, 128, 4], dt.float32, kind="ExternalInput").ap()
    bin_in = nc.dram_tensor("bin_", [L, 128, 2], dt.float32, kind="ExternalInput").ap()
    bhn_in = nc.dram_tensor("bhn", [L, 128, 2], dt.float32, kind="ExternalInput").ap()
    id_in = nc.dram_tensor("id128", [128, 128], dt.bfloat16, kind="ExternalInput").ap()
    out_t = nc.dram_tensor("outT", [2, 128, GPC], dt.float32, kind="ExternalOutput").ap()
    hdumps = []
    if debug:
        for i in range(L):
            hdumps.append(nc.dram_tensor(f"hd{i}", [128, 2, NB], dt.bfloat16,
                                         kind="ExternalOutput").ap())
        a_dump = nc.dram_tensor("adump", [GPC, 128, T, 2, WPG, WIN], dt.bfloat16,
                                kind="ExternalOutput").ap()
        m_dump = nc.dram_tensor("mdump", [GPC, 128, 2, MAXN], dt.bfloat16,
                                kind="ExternalOutput").ap()
        r_dump = nc.dram_tensor("rdump", [GPC, 2, 128, 2, 512], dt.float32,
                                kind="ExternalOutput").ap()
        z_dump = nc.dram_tensor("zdump", [GPC, 2, 128, 2, 512], dt.float32,
                                kind="ExternalOutput").ap()
        n_dump = nc.dram_tensor("nndump", [GPC, 2, 128, 2, 512], dt.float32,
                                kind="ExternalOutput").ap()

    groups = [list(range(NCORES))]

    with tile.TileContext(nc) as tc:
        with (
            tc.tile_pool(name="per", bufs=1) as per,       # persistent SBUF
            tc.tile_pool(name="wts", bufs=2) as wts,       # per-layer weights
            tc.tile_pool(name="gth", bufs=3) as gth,       # gather/S stream
            tc.tile_pool(name="wrk", bufs=2) as wrk,       # A/mT/staging
            tc.tile_pool(name="gru", bufs=6) as grup,      # GRU temps
            tc.tile_pool(name="ps", bufs=1, space="PSUM") as ps,
            tc.tile_pool(name="dram", bufs=2, space="DRAM") as dram,
        ):
            # persistent loads
            idx_sb = per.tile([128, SLOT16], dt.int16)
            nc.sync.dma_start(idx_sb[:], idx_in[:])
            counts_sb = per.tile([T, NB], dt.bfloat16)
            nc.sync.dma_start(counts_sb[:], counts_in[:])
            wp_sb = per.tile([128, 2, H], dt.bfloat16)
            nc.sync.dma_start(wp_sb[:], wp_in[:])
            bp_sb = per.tile([128, 2], dt.float32)
            nc.sync.dma_start(bp_sb[:], bp_in[:])
            id_sb = per.tile([128, 128], dt.bfloat16)
            nc.sync.dma_start(id_sb[:], id_in[:])
            xT_sb = per.tile([128, 2, NB], dt.bfloat16)
            nc.sync.dma_start(xT_sb[:], xT_in[:])
            hT_sb = per.tile([128, 2, NB], dt.bfloat16)
            outsb = per.tile([128, 2, GPC], dt.float32)
            nc.vector.memset(outsb[:], 0.0)

            # ---- input projection: hT = Wp @ xT + bp
            for hm in range(2):
                for s in range(NB // 512):
                    pm = ps.tile([128, 512], dt.float32, tag="mT", bufs=2)
                    nc.tensor.matmul(pm[:], wp_sb[:, 0, hm * 128:(hm + 1) * 128],
                                     xT_sb[:, 0, s * 512:(s + 1) * 512],
                                     start=True, stop=False)
                    nc.tensor.matmul(pm[:], wp_sb[:, 1, hm * 128:(hm + 1) * 128],
                                     xT_sb[:, 1, s * 512:(s + 1) * 512],
                                     start=False, stop=True)
                    nc.vector.tensor_scalar_add(hT_sb[:, hm, s * 512:(s + 1) * 512],
                                                pm[:], bp_sb[:, hm:hm + 1])

            if debug:
                nc.sync.dma_start(hdumps[0][:], hT_sb[:])
            rsums = {}
            for l in range(L):
                # ---- stage h (node-major) + AllGather into table
                stg = wrk.tile([128, NWIN, H], dt.bfloat16, tag="stg", bufs=1)
                if "tr" not in skip:
                    for w in range(NWIN):
                        for hc in range(2):
                            tp = ps.tile([128, 128], dt.bfloat16, tag="agg", bufs=2)
                            nc.tensor.transpose(tp[:], hT_sb[:, hc, w * 128:(w + 1) * 128],
                                                id_sb[:])
                            nc.scalar.copy(stg[:, w, hc * 128:(hc + 1) * 128], tp[:])
                agin = dram.tile([NB, H], dt.bfloat16, tag="agin")
                if "tr" not in skip:
                    nc.sync.dma_start(agin.rearrange("(w p) h -> p w h", p=128), stg[:])
                tbl = dram.tile([N, H], dt.bfloat16, tag="tbl", addr_space="Shared")
                if "ag" not in skip:
                    nc.gpsimd.collective_compute(
                        "AllGather", mybir.AluOpType.bypass, replica_groups=groups,
                        ins=[agin.opt()], outs=[tbl.opt()])
                else:
                    nc.sync.dma_start(tbl[0:NB], agin[:])

                # ---- layer weights
                wm_sb = wts.tile([128, 2, T, H], dt.bfloat16, tag="wm")
                nc.sync.dma_start(wm_sb[:], wm_in[l])
                bm_sb = wts.tile([T, H], dt.bfloat16, tag="bm")
                nc.sync.dma_start(bm_sb[:], bm_in[l])
                wih_sb = wts.tile([128, 2, 3 * H], dt.bfloat16, tag="wih")
                nc.sync.dma_start(wih_sb[:], wih_in[l])
                whh_sb = wts.tile([128, 2, 3 * H], dt.bfloat16, tag="whh")
                nc.sync.dma_start(whh_sb[:], whh_in[l])
                brz_sb = wts.tile([128, 4], dt.float32, tag="brz")
                nc.sync.dma_start(brz_sb[:], brz_in[l])
                bin_sb = wts.tile([128, 2], dt.float32, tag="bin")
                nc.sync.dma_start(bin_sb[:], bin_in[l])
                bhn_sb = wts.tile([128, 2], dt.float32, tag="bhn")
                nc.sync.dma_start(bhn_sb[:], bhn_in[l])

                # ---- aggregation + message + GRU, one graph (1024 nodes) at a time
                cglob = 0          # global chunk counter (program order)
                gg_tiles = {}      # gather-group -> (G, S)

                def need(c):
                    gg = c // 8
                    while len(gg_tiles) == 0 or max(gg_tiles) < gg:
                        g_ = 0 if not gg_tiles else max(gg_tiles) + 1
                        Gt = gth.tile([128, 8, H], dt.bfloat16, tag="G", name=f"G_{l}_{g_}")
                        if "gather" not in skip:
                            nc.gpsimd.dma_gather(
                                Gt[:], tbl[:], idx_sb[:, g_ * 64:(g_ + 1) * 64],
                                num_idxs=1024, num_idxs_reg=1024, elem_size=H)
                        else:
                            nc.sync.dma_start(
                                Gt[:], tbl[0:1024].rearrange("(c p) h -> p c h", p=128))
                        St = gth.tile([128, 8, WIN], dt.bfloat16, tag="S", name=f"S_{l}_{g_}")
                        if "sload" not in skip:
                            nc.sync.dma_start(St[:], smat_in[g_])
                        else:
                            nc.sync.dma_start(St[:], smat_in[0])
                        gg_tiles[g_] = (Gt, St)
                        if len(gg_tiles) > 4:
                            del gg_tiles[min(gg_tiles)]
                    return gg_tiles[gg], c % 8

                for q in range(GPC):
                    A_sb = wrk.tile([128, T, 2, WPG, WIN], dt.bfloat16, tag="A", bufs=1)
                    for wl in range(WPG):
                        w = q * WPG + wl
                        for th in range(T // 2):
                            pa = ps.tile([128, 512], dt.float32, tag="agg", bufs=2)
                            for ti in range(2):
                                t = th * 2 + ti
                                nchunks = int(budget[w, t])
                                for hc in range(2):
                                    off = (ti * 2 + hc) * 128
                                    for ci in range(nchunks):
                                        (Gt, St), j = need(cglob + ci)
                                        if "aggmm" in skip:
                                            continue
                                        nc.tensor.matmul(
                                            pa[:, off:off + 128],
                                            Gt[:, j, hc * 128:(hc + 1) * 128],
                                            St[:, j, :],
                                            start=(ci == 0), stop=(ci == nchunks - 1))
                                cglob += nchunks
                            dst_ap = A_sb[:, th * 2:th * 2 + 2, :, wl, :]
                            src_ap = pa.rearrange("p (t c k) -> p t c k", t=2, c=2)
                            if "aggcp" not in skip:
                                if th % 2 == 0:
                                    nc.scalar.copy(dst_ap, src_ap)
                                else:
                                    nc.vector.tensor_copy(dst_ap, src_ap)

                    if debug and l == 0:
                        nc.sync.dma_start(a_dump[q], A_sb[:])
                    # ---- message matmuls: mT = sum_t WmT[t] @ A_t + bm @ counts
                    mT_sb = wrk.tile([128, 2, MAXN], dt.bfloat16, tag="mT")
                    for hm in range(2):
                        for s2 in range(2):
                            pm = ps.tile([128, 512], dt.float32, tag="mT", bufs=2)
                            nbase = q * MAXN + s2 * 512
                            if "wt" not in skip:
                                nc.tensor.matmul(
                                    pm[:], bm_sb[:, hm * 128:(hm + 1) * 128],
                                    counts_sb[:, nbase:nbase + 512],
                                    start=True, stop=False)
                                for t in range(T):
                                    for hk in range(2):
                                        nc.tensor.matmul(
                                            pm[:],
                                            wm_sb[:, hk, t, hm * 128:(hm + 1) * 128],
                                            A_sb[:, t, hk, s2 * 4:(s2 + 1) * 4, :],
                                            start=False, stop=(t == T - 1 and hk == 1))
                                nc.vector.tensor_copy(mT_sb[:, hm, s2 * 512:(s2 + 1) * 512], pm[:])

                    if debug and l == 0:
                        nc.sync.dma_start(m_dump[q], mT_sb[:])
                    # ---- GRU for this graph's 1024 nodes, in 512-slices
                    for hq in range(2):
                        if "gru" in skip:
                            continue
                        nbase = q * MAXN + hq * 512
                        nsl = slice(nbase, nbase + 512)
                        msl = slice(hq * 512, (hq + 1) * 512)
                        r_sb = grup.tile([128, 2, 512], dt.float32, tag="r", bufs=2)
                        z_sb = grup.tile([128, 2, 512], dt.float32, tag="z", bufs=2)
                        for gm in range(4):
                            pg = ps.tile([128, 512], dt.float32, tag="gru", bufs=3)
                            gsl = slice(gm * 128, (gm + 1) * 128)
                            nc.tensor.matmul(pg[:], wih_sb[:, 0, gsl], mT_sb[:, 0, msl],
                                             start=True, stop=False)
                            nc.tensor.matmul(pg[:], wih_sb[:, 1, gsl], mT_sb[:, 1, msl],
                                             start=False, stop=False)
                            nc.tensor.matmul(pg[:], whh_sb[:, 0, gsl], hT_sb[:, 0, nsl],
                                             start=False, stop=False)
                            nc.tensor.matmul(pg[:], whh_sb[:, 1, gsl], hT_sb[:, 1, nsl],
                                             start=False, stop=True)
                            dst = r_sb[:, gm, :] if gm < 2 else z_sb[:, gm - 2, :]
                            nc.scalar.activation(dst, pg[:],
                                                 mybir.ActivationFunctionType.Sigmoid,
                                                 bias=brz_sb[:, gm:gm + 1])
                        if debug and l == 0:
                            nc.sync.dma_start(r_dump[q, hq], r_sb[:])
                            nc.sync.dma_start(z_dump[q, hq], z_sb[:])
                        nns, zds = [], []
                        for hc in range(2):
                            gsl = slice((4 + hc) * 128, (5 + hc) * 128)
                            ph = ps.tile([128, 512], dt.float32, tag="gru", bufs=3)
                            nc.tensor.matmul(ph[:], whh_sb[:, 0, gsl], hT_sb[:, 0, nsl],
                                             start=True, stop=False)
                            nc.tensor.matmul(ph[:], whh_sb[:, 1, gsl], hT_sb[:, 1, nsl],
                                             start=False, stop=True)
                            hnb = grup.tile([128, 512], dt.float32, tag="gt", bufs=4)
                            nc.vector.tensor_scalar_add(hnb[:], ph[:], bhn_sb[:, hc:hc + 1])
                            rhn = grup.tile([128, 512], dt.float32, tag="gt", bufs=4)
                            nc.vector.tensor_mul(rhn[:], r_sb[:, hc, :], hnb[:])
                            pi = ps.tile([128, 512], dt.float32, tag="gru", bufs=3)
                            nc.tensor.matmul(pi[:], wih_sb[:, 0, gsl], mT_sb[:, 0, msl],
                                             start=True, stop=False)
                            nc.tensor.matmul(pi[:], wih_sb[:, 1, gsl], mT_sb[:, 1, msl],
                                             start=False, stop=True)
                            tsum = grup.tile([128, 512], dt.float32, tag="gt", bufs=4)
                            nc.vector.tensor_add(tsum[:], pi[:], rhn[:])
                            nn = grup.tile([128, 512], dt.float32, tag="nnb", bufs=3)
                            nc.scalar.activation(nn[:], tsum[:],
                                                 mybir.ActivationFunctionType.Tanh,
                                                 bias=bin_sb[:, hc:hc + 1])
                            if debug and l == 0:
                                nc.sync.dma_start(n_dump[q, hq, :, hc, :], nn[:])
                            hprev = grup.tile([128, 512], dt.float32, tag="gt", bufs=4)
                            nc.vector.tensor_copy(hprev[:], hT_sb[:, hc, nsl])
                            d_ = grup.tile([128, 512], dt.float32, tag="gt", bufs=4)
                            nc.vector.tensor_sub(d_[:], hprev[:], nn[:])
                            zd = grup.tile([128, 512], dt.float32, tag="zdb", bufs=3)
                            nc.vector.tensor_mul(zd[:], z_sb[:, hc, :], d_[:])
                            nns.append(nn)
                            zds.append(zd)
                        # write h only after BOTH halves' matmuls consumed h_l
                        for hc in range(2):
                            if l < L - 1:
                                nc.vector.tensor_add(hT_sb[:, hc, nsl], nns[hc][:], zds[hc][:])
                            else:
                                hf = grup.tile([128, 512], dt.float32, tag="hf", bufs=2)
                                nc.vector.tensor_add(hf[:], nns[hc][:], zds[hc][:])
                                rs = grup.tile([128, 1], dt.float32, tag="rs", bufs=16)
                                nc.vector.tensor_reduce(rs[:], hf[:],
                                                        axis=mybir.AxisListType.X,
                                                        op=mybir.AluOpType.add)
                                rsums[(q, hc, hq)] = rs
                assert cglob == int(budget.sum()), (cglob, int(budget.sum()))
                if debug and l < L - 1:
                    nc.sync.dma_start(hdumps[l + 1][:], hT_sb[:])

            # ---- readout
            for q in range(GPC):
                for hc in range(2):
                    if (q, hc, 0) not in rsums:
                        continue
                    nc.vector.tensor_add(outsb[:, hc, q:q + 1],
                                         rsums[(q, hc, 0)][:], rsums[(q, hc, 1)][:])
            nc.sync.dma_start(out_t.rearrange("c p g -> p c g"), outsb[:])

    nc.compile()
    return nc


def kernel(**inputs):
    meta, in_maps = _prep(**inputs)
    nc = _build(meta)
    res = run_bass_kernel_spmd(nc, in_maps, core_ids=list(range(NCORES)))
    GPC = meta["GPC"]
    out = np.zeros((meta["B"], H), np.float32)
    for c in range(NCORES):
        ot = res.results[c]["outT"]          # [2, 128, GPC]
        for g in range(GPC):
            out[c * GPC + g] = np.concatenate([ot[0, :, g], ot[1, :, g]])
    return out

